# revision 7
# baseline (speedup 1.0000x reference)
"""DeltaSynapse kernel for Trainium2 (8 NeuronCores, SPMD).

Reference computation:
    Xpre[b,e,o] = sum_d delaymap[d,e,o] * Xd[d,b,e]
    I[b,o]      = sum_e (signs*W)[e,o] * Xpre[b,e,o]

Folded:  I[b,o] = sum_{d,e} (delaymap[d,e,o] * Weff[e,o]) * Xd[d,b,e]
i.e. a sum of D=8 matmuls I += Xd[d] @ (delaymap[d] . Weff).

delaymap is a one-hot over the 8 delays: pure structure, 3 bits per
synapse, which the baseline streamed as 128 MiB of fp32. This kernel
compresses it on the host into bit-planes of didx = argmax_d delaymap.
With hi = didx>>2, l0 = didx&1, l1 = (didx>>1)&1, W_a = Weff*(hi==a),
and q_a = W_a*l0 (multilinear expansion of the one-hot in the low two
index bits; X-side combinations are tiny and host-precomputed):

  I = sum_a [ Xd[4a]             @ W_a
            + (Xd[4a+1]-Xd[4a])  @ q_a
            + (Xd[4a+2]-Xd[4a])  @ (W_a . l1)
            + (Xd[4a+3]-Xd[4a+2]-Xd[4a+1]+Xd[4a]) @ (q_a . l1) ]

HBM traffic per core drops ~18 MiB -> ~4.8 MiB: four fp16 planes
(W0, W1, q0, q1; l0 already folded on host) + one fp8 l1 plane (exact:
values 0/1), all e-sliced. The device rebuilds the four l1-masked
planes with TWO fused DVE multiplies per o-range (dual-plane +
dual-chunk in one instruction, l1 broadcast over the plane axis;
everything fp16 unit-stride so DVE runs its 2x packed mode), then runs
the usual 16-matmul PSUM accumulation per range.

Engine budget per core (trace-calibrated): PE 32768 matmul rows fp16
~13.7us at full clock (+DVFS ramp-up: the tensor engine runs ~1.2 GHz
until ~3us of continuous work, so the schedule avoids PE gaps), DMA
~4.8 MiB at ~342 GB/s ~14us, DVE ~10us, Pool 8 SWDGE descriptor-gens
~8us, Act psum->sbuf copies. PE/DMA co-bound.

Scheduling notes (from perfetto traces of prior revisions):
  - HWDGE (sync) transfers starve at ~14 GB/s while the SWDGE queue
    streams, so only tensors needed in the first ~1.5us (yc, wl0) ride
    HWDGE -- they finish before the SWDGE stream ramps. Everything
    else goes on the one SWDGE queue in consumption order.
  - SWDGE descriptor-gen occupies the Pool engine ~1us per dma_start:
    keep the count low (8) and keep Pool otherwise idle.
  - Pool and DVE running tensor ops concurrently on the same tiles
    slow each other ~3x (SBUF contention) -> all products on DVE.
  - o-ranges ramp up then taper so the DMA stream stays ahead of PE
    (PE idle gaps reset the DVFS clock) and the final range's
    matmul+copy+store tail is short.

Sharding: contraction (pre-neuron e) dim across 8 cores, 256 rows
each; every core emits a full [16, 2048] partial, host sums.
"""

import numpy as np

D, B, N = 8, 16, 2048
NCORES = 8
P = 128                 # SBUF partitions / matmul contraction tile
ESH = N // NCORES       # per-core pre-dim shard = 256
ECH = ESH // P          # e-chunks per core = 2
O_WIDTHS = [128, 448, 512, 512, 384, 64]
O_RANGES = []
_o = 0
for _w in O_WIDTHS:
    O_RANGES.append((_o, _o + _w))
    _o += _w
assert _o == N
NR = len(O_RANGES)
LAMA = O_WIDTHS[0]      # l1 cols arriving early (range 0)
TAILN = 2               # last ranges share one output tile + DMA

_prog_cache = {}


def _build_program():
    from concourse import bacc, tile
    from concourse import mybir

    f32 = mybir.dt.float32
    f16 = mybir.dt.float16
    f8 = mybir.dt.float8e4

    nc = bacc.Bacc(enable_partition_id=False)
    # Host-prepared layouts (see _shard_inputs):
    #   wl{r}: [P, ECH, 4, w_r] f16  planes (W0, W1, q0, q1), o-range r
    #   lama : [P, ECH, LAMA]   f8   l1 plane, ranges 0-1
    #   lamb : [P, ECH, N-LAMA] f8   l1 plane, remaining ranges
    #   yc   : [P, ECH, 8, B]   f16  X-side multilinear combos
    wls = {}
    for r, (o0, o1) in enumerate(O_RANGES):
        wls[r] = nc.dram_tensor(f"wl{r}", [P, ECH, 4, o1 - o0], f16,
                                kind="ExternalInput")
    lama_d = nc.dram_tensor("lama", [P, ECH, LAMA], f8, kind="ExternalInput")
    lamb_d = nc.dram_tensor("lamb", [P, ECH, N - LAMA], f8,
                            kind="ExternalInput")
    ycd = nc.dram_tensor("yc", [P, ECH, 8, B], f16, kind="ExternalInput")
    out = nc.dram_tensor("out", [B, N], f32, kind="ExternalOutput")

    with tile.TileContext(nc) as tc:
        with (
            tc.tile_pool(name="const", bufs=1) as cpool,
            tc.tile_pool(name="wl", bufs=NR) as wlpool,
            tc.tile_pool(name="wd", bufs=3) as wdpool,
            tc.tile_pool(name="psum", bufs=7, space="PSUM") as ppool,
            tc.tile_pool(name="outp", bufs=7) as opool,
        ):
            yc = cpool.tile([P, ECH, 8, B], f16)
            lama = cpool.tile([P, ECH, LAMA], f16)
            lamb = cpool.tile([P, ECH, N - LAMA], f16)
            wl_tiles = {}
            for r, (o0, o1) in enumerate(O_RANGES):
                wl_tiles[r] = wlpool.tile([P, ECH, 4, o1 - o0], f16,
                                          tag="wl", name=f"wl{r}")

            # Early small tensors on HWDGE (finish before the SWDGE
            # stream ramps and starves this queue); the main stream on
            # SWDGE in consumption order (fp8 l1 is SWDGE-cast to fp16).
            nc.sync.dma_start(yc[:], ycd[:])
            nc.sync.dma_start(wl_tiles[0][:], wls[0][:])
            nc.gpsimd.dma_start(lama[:], lama_d[:])
            nc.gpsimd.dma_start(wl_tiles[1][:], wls[1][:])
            nc.gpsimd.dma_start(wl_tiles[2][:], wls[2][:])
            nc.gpsimd.dma_start(lamb[:], lamb_d[:])
            for r in range(3, NR):
                nc.gpsimd.dma_start(wl_tiles[r][:], wls[r][:])

            tail0 = NR - TAILN
            t_o0 = O_RANGES[tail0][0]
            tail_t = opool.tile([B, N - t_o0], f32, tag="otail")

            for r, (o0, o1) in enumerate(O_RANGES):
                w = o1 - o0
                psum = ppool.tile([B, 512], f32, tag="ps", name=f"ps{r}")
                wl = wl_tiles[r]
                if r < 1:
                    lam = lama[:, :, o0:o1]
                else:
                    lam = lamb[:, :, o0 - LAMA:o1 - LAMA]
                lam_b2 = lam.unsqueeze(2).broadcast_to([P, ECH, 2, w])
                # rebuild l1-masked planes: (s0,s1) = (W0,W1).l1 and
                # (t0,t1) = (q0,q1).l1 -- one fused DVE mult per pair,
                # both chunks at once; l1 in {0,1} keeps them exact.
                wd = wdpool.tile([P, ECH, 4, 512], f16, tag="wd")
                nc.vector.tensor_mul(wd[:, :, 0:2, :w], wl[:, :, 0:2, :],
                                     lam_b2)
                nc.vector.tensor_mul(wd[:, :, 2:4, :w], wl[:, :, 2:4, :],
                                     lam_b2)
                # direct planes first (depend only on DMA), product
                # planes after (DVE runs a range ahead of PE)
                for c in range(ECH):
                    for j in range(4):
                        nc.tensor.matmul(psum[:, :w], yc[:, c, j, :],
                                         wl[:, c, j, :],
                                         start=(c == 0 and j == 0),
                                         stop=False)
                for c in range(ECH):
                    for j in range(4):
                        nc.tensor.matmul(psum[:, :w], yc[:, c, 4 + j, :],
                                         wd[:, c, j, :w],
                                         start=False,
                                         stop=(c == ECH - 1 and j == 3))
                # o-range complete: stream it out
                if r < tail0:
                    out_t = opool.tile([B, 512], f32, tag="out", name=f"o{r}")
                    nc.scalar.copy(out_t[:, :w], psum[:, :w])
                    nc.sync.dma_start(out[:, o0:o1], out_t[:, :w])
                else:
                    nc.scalar.copy(tail_t[:, o0 - t_o0:o1 - t_o0], psum[:, :w])
                    if r == NR - 1:
                        nc.sync.dma_start(out[:, t_o0:], tail_t[:])

    nc.compile()
    return nc


def _get_program():
    if "nc" not in _prog_cache:
        _prog_cache["nc"] = _build_program()
    return _prog_cache["nc"]


def _shard_inputs(Xd, delaymap, W, signs):
    """Compress delaymap to bit-planes, build per-core fp16 input maps."""
    import ml_dtypes

    Xd = np.asarray(Xd, dtype=np.float32)
    delaymap = np.asarray(delaymap, dtype=np.float32)
    W = np.asarray(W, dtype=np.float32)
    signs = np.asarray(signs, dtype=np.float32)

    didx = np.argmax(delaymap, axis=0).astype(np.uint8)     # (N, N) in [0,8)
    Weff = signs * W
    hi = didx >> 2
    l0 = (didx & 1).astype(np.float32)
    W0 = np.where(hi == 0, Weff, 0.0)
    W1 = Weff - W0
    planes = np.empty((4, N, N), dtype=np.float16)
    planes[0] = W0
    planes[1] = W1
    planes[2] = W0 * l0                                     # q0
    planes[3] = W1 * l0                                     # q1
    l1 = ((didx >> 1) & 1).astype(ml_dtypes.float8_e4m3fn)  # exact 0/1

    in_maps = []
    for k in range(NCORES):
        esl = slice(k * ESH, (k + 1) * ESH)
        # [4, ESH, N] -> [P, ECH, 4, N] (e = c*128 + p), then o-range slices
        pl = planes[:, esl, :].reshape(4, ECH, P, N).transpose(2, 1, 0, 3)
        m = {}
        for r, (o0, o1) in enumerate(O_RANGES):
            m[f"wl{r}"] = np.ascontiguousarray(pl[:, :, :, o0:o1])
        lam = l1[esl].reshape(ECH, P, N).transpose(1, 0, 2)  # [P, ECH, N]
        m["lama"] = np.ascontiguousarray(lam[:, :, :LAMA])
        m["lamb"] = np.ascontiguousarray(lam[:, :, LAMA:])
        # X-side multilinear combos, lhsT order j = (subset, a):
        #   [X0, X4, X1-X0, X5-X4, X2-X0, X6-X4, X3-X2-X1+X0, X7-X6-X5+X4]
        xe = Xd[:, :, esl]                                  # (D, B, ESH)
        Y = np.empty((8, B, ESH), dtype=np.float32)
        for a in (0, 1):
            b4 = xe[4 * a:4 * a + 4]
            Y[0 + a] = b4[0]
            Y[2 + a] = b4[1] - b4[0]
            Y[4 + a] = b4[2] - b4[0]
            Y[6 + a] = b4[3] - b4[2] - b4[1] + b4[0]
        m["yc"] = np.ascontiguousarray(
            Y.reshape(8, B, ECH, P).transpose(3, 2, 0, 1).astype(np.float16)
        )
        in_maps.append(m)
    return in_maps


def _run(in_maps, trace=False, **kw):
    from concourse.bass_utils import run_bass_kernel_spmd

    nc = _get_program()
    return run_bass_kernel_spmd(nc, in_maps, list(range(NCORES)), trace=trace, **kw)


def _gather(res):
    acc = np.zeros((B, N), dtype=np.float64)
    for k in range(NCORES):
        acc += res.results[k]["out"].astype(np.float64)
    return acc.astype(np.float32)


def kernel(Xd, X, delaymap, W, signs):
    in_maps = _shard_inputs(Xd, delaymap, W, signs)
    return _gather(_run(in_maps))
